# revision 2
# baseline (speedup 1.0000x reference)
"""Trainium2 Bass kernel for the HPM gaussian-ray read problem (sparse, v2).

out[b,c] = sum_n exp(-r2[n,b]/(2*sigma^2)) * exp(-max(t[n,b],0)/tau) * mem[n,c]

over the flattened 128^3 grid, B=32 rays, C=16 channels.

v2 design (vs the 32-z baseline): 16-z entries, fully batched matmuls.

Host classifies each (column, ray) pair (inactive / branch W0 / branch W1 /
straddle) exactly as before.  Active columns are grouped into *entries*
(column, zb) covering a 16-z window.  Entries and their pair-slots are
packed into NM=22 batched matmuls; each matmul m:

  mm2_m: psO[64 slots, 128] = kern_m[128, 64].T @ mem_m[128, 128]

where the 128-row contraction dim is 8 sub-tiles x 16 z rows, and the
128 moving cols are 8 entry positions x 16 channels.  kern_m is made
block-diagonal (slot s only sees its entry's 16 z rows) through 8
indicator rows in the basis: coef adds WNEG (-> exp == 0) on the 7 wrong
sub-tiles.  Straddle pairs use 15 extra step-basis rows [z >= k] to
split exactly at the t=0 kink into a W0-slot and a W1-slot, so there is
no separate straddle path at all.

Basis (31 rows): [C0,C1,C2, B0,B1,B2, A0,A1] split-bf16 quadratic rows,
8 q-indicator rows, 15 z-step rows.

Device program per core (all cores run the same static program):
  - DMA in: zaugD [31,128], coef [31, NM*64], mem [128, NM*128] (3 queues)
  - mm1 x3: psW bank b [128, <=512] = zaugD.T @ coef[:, 512b:...]
  - Exp x3 (ACT): kern[:, bank] = exp(psW_b)  (bf16)
  - mm2 x NM into 3 psO PSUM banks, alternating tile_position (0,0)/(0,64)
  - 3 evacuation copies psO -> outbuf (bf16) + 3 out DMAs
Host gathers the per-slot 16-channel rows and scatter-adds into out[B,C].
"""

import numpy as np

SIGMA = 0.5
TAU = 2.0
NCORES = 8
D = 128           # grid edge
B = 32            # rays
C = 16            # channels
NCH = D * D       # 16384 (gx,gy) columns

ZS = 16           # z rows per entry
NQ = 8            # sub-tiles (z blocks) per matmul
NJ = 8            # entry positions per sub-tile... (shared col space: 8 j's)
EPM = 64          # entry positions per matmul (8 q x 8 j)
SPM = 64          # slots per matmul
NM = 22           # matmuls per core (static capacity; psW banks 512/512/384)
PB = (NM + 1) // 2            # psO pair-blocks (11)
BANKW = [512, 512, 384]       # slot cols per psW/coef bank
RQUAD = 8                     # quad basis rows
RIND = 8                      # q-indicator rows
RSTEP = 15                    # z-step rows [z >= k], k=1..15
R = RQUAD + RIND + RSTEP      # 31 basis rows
EPS = 1e-4        # per-pair neglected-mass threshold
YTHR = 1e-8       # per-z weight threshold for z-windows
WNEG = -30000.0   # "minus infinity" log-weight (bf16 exact: -29952)

_BASS_CACHE = {}


# ---------------------------------------------------------------- device ---

def _build_nc():
    from contextlib import ExitStack
    import concourse.bacc as bacc
    import concourse.mybir as mybir

    f32 = mybir.dt.float32
    bf16 = mybir.dt.bfloat16
    nc = bacc.Bacc()
    # coef: basis rows of psW bank b live at partitions 32b..32b+R-1,
    # cols 0..511 are bank b's slot columns.  zaug: 4 stacked copies of
    # the [R, 128] basis at partition offsets 0/32/64/96.  128-partition
    # layouts keep the DMA descriptors spread across all engines.
    # zc: cols 0:128 = zaug basis (4 stacked copies), cols 128:640 = coef
    # (bank b rows at partitions 32b..32b+R-1) -- one DMA, one gate
    zc_d = nc.dram_tensor("zc", [128, 640], bf16, kind="ExternalInput")
    mem_d = nc.dram_tensor("mem", [128, NM * 128], bf16, kind="ExternalInput")
    out_d = nc.dram_tensor("out", [128, PB * 128], bf16, kind="ExternalOutput")
    Exp = mybir.ActivationFunctionType.Exp

    MEMC = NM * 128                        # 3072

    with ExitStack() as ctx:
        zcsb = ctx.enter_context(nc.sbuf_tensor("zcsb", [128, 640], bf16))
        memsb = ctx.enter_context(nc.sbuf_tensor("memsb", [128, MEMC], bf16))
        kern = ctx.enter_context(nc.sbuf_tensor("kern", [128, NM * SPM],
                                                bf16))
        outbuf = ctx.enter_context(
            nc.sbuf_tensor("outbuf", [128, PB * 128], bf16))
        psw = [ctx.enter_context(
            nc.psum_tensor(f"psw{b}", [128, BANKW[b]], f32))
            for b in range(3)]
        PSOW = [512, 512, PB * 128 - 1024]
        pso = [ctx.enter_context(
            nc.psum_tensor(f"pso{b}", [128, PSOW[b]], f32))
            for b in range(3)]

        s_zc = nc.alloc_semaphore("s_zc")
        s_m = [nc.alloc_semaphore(f"s_m{i}") for i in range(3)]
        s_w = nc.alloc_semaphore("s_w")
        s_k = nc.alloc_semaphore("s_k")
        s_pso = nc.alloc_semaphore("s_pso")
        s_cpv = nc.alloc_semaphore("s_cpv")
        s_fin = [nc.alloc_semaphore(f"s_fin{i}") for i in range(3)]
        s_cps = nc.alloc_semaphore("s_cps")

        # --- DMA issue (async; per-ring in-order) ---
        # sync: one zaug+coef transfer (mm1 gate); gpsimd/scalar: memory
        nc.sync.dma_start(out=zcsb[:], in_=zc_d[:, :]).then_inc(s_zc, 16)
        nc.gpsimd.dma_start(out=memsb[:, 0:1024],
                            in_=mem_d[:, 0:1024]).then_inc(s_m[0], 16)
        nc.gpsimd.dma_start(out=memsb[:, 1024:2048],
                            in_=mem_d[:, 1024:2048]).then_inc(s_m[1], 16)
        nc.scalar.dma_start(out=memsb[:, 2048:MEMC],
                            in_=mem_d[:, 2048:MEMC]).then_inc(s_m[2], 16)

        # --- tensor: 3 x mm1 (separate PE row-tiles), 22 x mm2 ---
        nc.tensor.wait_ge(s_zc, 16)
        for b in range(3):
            nc.tensor.matmul(psw[b][:, 0:BANKW[b]],
                             zcsb[32 * b:32 * b + R, 0:128],
                             zcsb[32 * b:32 * b + R, 128:128 + BANKW[b]],
                             start=True, stop=True,
                             tile_position=(32 * b, 0)).then_inc(s_w, 1)
        # s_pso ticks after each psO bank completes (full-bank casts only:
        # reading a PSUM bank while the PE still writes other columns of
        # the same bank wedges real hardware)
        PSO_TICKS = (7, 15, 21)
        for m in range(NM):
            if m == 0:
                nc.tensor.wait_ge(s_k, 1)
                nc.tensor.wait_ge(s_m[0], 16)
            elif m == 8:
                nc.tensor.wait_ge(s_k, 2)
                nc.tensor.wait_ge(s_m[1], 16)
            elif m == 16:
                nc.tensor.wait_ge(s_k, 3)
                nc.tensor.wait_ge(s_m[2], 16)
            p = m // 2
            r0 = 64 * (m % 2)
            c0 = 128 * (p % 4)
            mm = nc.tensor.matmul(
                pso[p // 4][r0:r0 + 64, c0:c0 + 128],
                kern[:, SPM * m:SPM * (m + 1)],
                memsb[:, 128 * m:128 * (m + 1)],
                start=True, stop=True,
                tile_position=(0, r0))
            if m in PSO_TICKS:
                mm.then_inc(s_pso, 1)

        # --- scalar (ACT): 3 x Exp, then bank-2 casts + its out DMA ---
        co = 0
        for b in range(3):
            nc.scalar.wait_ge(s_w, b + 1)
            nc.scalar.activation(kern[:, co:co + BANKW[b]],
                                 psw[b][:, 0:BANKW[b]], Exp).then_inc(s_k, 1)
            co += BANKW[b]
        nc.scalar.wait_ge(s_pso, 3)
        nc.scalar.copy(out=outbuf[:, 1024:1408],
                       in_=pso[2][:, 0:384]).then_inc(s_cps, 1)
        nc.scalar.wait_ge(s_cps, 1)
        nc.scalar.dma_start(out=out_d[:, 1024:1408],
                            in_=outbuf[:, 1024:1408]).then_inc(s_fin[2], 16)

        # --- vector (DVE): full-bank casts of pso banks 0/1 ---
        for h in range(2):
            nc.vector.wait_ge(s_pso, h + 1)
            nc.vector.tensor_copy(
                outbuf[:, 512 * h:512 * (h + 1)],
                pso[h][:, 0:512],
            ).then_inc(s_cpv, 1)

        # --- out DMAs chase the casts on sync / gpsimd rings ---
        nc.sync.wait_ge(s_cpv, 1)
        nc.sync.dma_start(out=out_d[:, 0:512],
                          in_=outbuf[:, 0:512]).then_inc(s_fin[0], 16)
        nc.gpsimd.wait_ge(s_cpv, 2)
        nc.gpsimd.dma_start(out=out_d[:, 512:1024],
                            in_=outbuf[:, 512:1024]).then_inc(s_fin[1], 16)

        nc.compile()
    return nc


def _get_nc():
    if "nc" not in _BASS_CACHE:
        _BASS_CACHE["nc"] = _build_nc()
    return _BASS_CACHE["nc"]


# ------------------------------------------------------------------ host ---

def _bf16(x):
    import ml_dtypes
    return np.asarray(x).astype(ml_dtypes.bfloat16)


def _split3(x):
    """f64 -> three bf16 parts summing to ~24 mantissa bits of x."""
    x0 = _bf16(x).astype(np.float64)
    x1 = _bf16(x - x0).astype(np.float64)
    x2 = _bf16(x - x0 - x1).astype(np.float64)
    return x0, x1, x2


def _pack_cols(Aq, Bq, Cq, zb):
    """f64 quadratic in u_orig = z - 64 -> [8, n] bf16 split rows,
    recentered to u = z - zb - 8.  Rows: [C0,C1,C2, B0,B1,B2, A0,A1]."""
    Aq = np.asarray(Aq, np.float64)
    Bq = np.asarray(Bq, np.float64)
    Cq = np.asarray(Cq, np.float64)
    zb = np.asarray(zb, np.float64)
    cs = zb + 8.0 - 64.0
    Bt = Bq + 2 * Aq * cs
    Ct = Cq + Bq * cs + Aq * cs * cs
    C_0, C_1, C_2 = _split3(Ct)
    B_0, B_1, B_2 = _split3(Bt)
    A_0, A_1, _ = _split3(Aq)
    rows = [C_0, C_1, C_2, B_0, B_1, B_2, A_0, A_1]
    return np.stack([_bf16(r) for r in rows])


def _zaug_rows():
    """Basis [128, 128]: 4 stacked copies (partition offsets 0/32/64/96)
    of the [31, 128] basis over {q,z} cols: quad rows on u = z - 8,
    8 q-indicator rows, 15 step rows [z >= k]."""
    u = np.arange(ZS, dtype=np.float64) - 8.0
    one = np.ones(ZS)
    quad = np.stack([one, one, one, u, u, u, u * u, u * u])      # [8, 16]
    base = np.zeros((R, 128), np.float64)
    for q in range(NQ):
        base[0:RQUAD, ZS * q:ZS * (q + 1)] = quad
        base[RQUAD + q, ZS * q:ZS * (q + 1)] = 1.0
        for k in range(1, 16):
            base[RQUAD + RIND + k - 1, ZS * q + k:ZS * (q + 1)] = 1.0
    out = np.zeros((128, 128), np.float64)
    for b in range(4):
        out[32 * b:32 * b + R] = base
    return _bf16(out)


def _analyze(ray_origin, ray_dir):
    """Quadratic coeffs (f64) + per-(col, ray) branch assignment.

    assign: 0=inactive, 1=W0 branch, 2=W1 branch, 3=straddle."""
    o = ray_origin.astype(np.float64)
    d = ray_dir.astype(np.float64)
    d2 = (d * d).sum(-1)
    kap = 2.0 - d2
    od = (o * d).sum(-1)
    g = np.arange(D, dtype=np.float64)
    gx = np.repeat(g, D)
    gy = np.tile(g, D)
    c1 = 1.0 / (2 * SIGMA ** 2)
    c3 = 1.0 / TAU
    alpha = gx[:, None] * d[None, :, 0] + gy[:, None] * d[None, :, 1] - od[None, :]
    t64 = 64.0 * d[None, :, 2] + alpha                      # [NCH, B]
    e = 64.0 - o[:, 2]
    gamma = (gx[:, None] - o[None, :, 0]) ** 2 + (gy[:, None] - o[None, :, 1]) ** 2
    A0 = np.broadcast_to((-c1 + c1 * kap * d[:, 2] ** 2)[None, :], t64.shape)
    B0 = -2 * c1 * e[None, :] + 2 * c1 * kap[None, :] * d[None, :, 2] * t64
    C0 = -c1 * (gamma + e[None, :] ** 2) + c1 * kap[None, :] * t64 ** 2
    B1 = B0 - c3 * d[None, :, 2]
    C1 = C0 - c3 * t64

    u = np.arange(D, dtype=np.float64) - 64.0
    assign = np.zeros((NCH, B), np.int8)
    lo = np.full((NCH, B), D - 1, np.int32)
    hi = np.zeros((NCH, B), np.int32)
    CH = 2048
    for s in range(0, NCH, CH):
        sl = slice(s, s + CH)
        W0 = (A0[sl, :, None] * u[None, None, :] ** 2
              + B0[sl, :, None] * u[None, None, :] + C0[sl, :, None])
        W1 = (A0[sl, :, None] * u[None, None, :] ** 2
              + B1[sl, :, None] * u[None, None, :] + C1[sl, :, None])
        y0 = np.exp(np.minimum(W0, 50.0))
        y1 = np.exp(np.minimum(W1, 50.0))
        yt = np.minimum(y0, y1)
        Ec = yt.sum(-1)
        E0 = (y0 - yt).sum(-1)
        E1 = (y1 - yt).sum(-1)
        a = np.full(Ec.shape, 3, np.int8)
        a[E1 <= EPS] = 2
        a[E0 <= EPS] = 1
        a[Ec <= EPS] = 0
        assign[sl] = a
        m = yt >= YTHR
        any_ = m.any(-1)
        lo[sl] = np.where(any_, m.argmax(-1), D - 1)
        hi[sl] = np.where(any_, D - 1 - m[:, :, ::-1].argmax(-1), 0)
    return assign, lo, hi, (A0, B0, C0, B1, C1), alpha


def _col_entries(col, rays, lo, hi):
    """(col, zb, rays-subset) entries with disjoint 16-z coverage."""
    clo = int(lo[col, rays].min())
    chi = int(hi[col, rays].max())
    if chi - clo + 1 <= ZS:
        zb = min(clo, D - ZS)
        return [(col, zb, list(rays))]
    out = []
    for k in range(clo // ZS, chi // ZS + 1):
        rs = [r for r in rays
              if lo[col, r] < ZS * (k + 1) and hi[col, r] >= ZS * k]
        if rs:
            out.append((col, ZS * k, rs))
    return out


def _plan(assign, lo, hi, alpha, dz):
    """Columns -> cores, entries -> matmuls.

    Returns per-core list of matmuls; each matmul is a list of
    (col, zb, ent_idx, [slots]) where each slot is
    (ray, branch, theta, resurrect):
      branch 0 -> (B0, C0);  branch 1 -> (B1, C1)
      theta None -> plain slot;  else step at z >= theta within the entry,
      resurrect False -> slot active on z < theta (kill upper),
      resurrect True  -> active on z >= theta (kill lower).
    """
    act = assign != 0
    # per-column slot load (straddle pairs may need 2 slots)
    slots_per_col = ((assign == 1) | (assign == 2)).sum(1) \
        + 2 * (assign == 3).sum(1)
    cols = np.nonzero(act.any(1))[0]
    order = cols[np.argsort(-slots_per_col[cols], kind="stable")]
    loads = np.zeros(NCORES, np.int64)
    colcnt = np.zeros(NCORES, np.int64)
    core_cols = [[] for _ in range(NCORES)]
    for col in order:
        k = int(np.lexsort((colcnt, loads))[0])
        core_cols[k].append(col)
        loads[k] += slots_per_col[col]
        colcnt[k] += 1

    plans = []
    for k in range(NCORES):
        entries = []     # (col, zb, [slotspec])
        for col in sorted(core_cols[k]):
            rays = np.nonzero(act[col])[0]
            for ecol, zb, rs in _col_entries(col, rays, lo, hi):
                slots = []
                for r in rs:
                    a = assign[col, r]
                    if a == 1:
                        slots.append((int(r), 0, None, False))
                    elif a == 2:
                        slots.append((int(r), 1, None, False))
                    else:
                        # straddle: split at the t=0 kink z* = -alpha/dz
                        dzr = dz[r]
                        zstar = -alpha[col, r] / dzr if dzr != 0 else 1e9
                        zz = zstar - zb
                        th = int(np.ceil(zz))
                        # upper region z >= z*: t>0 (W1) if dz>0 else t<=0
                        up_br = 1 if dzr > 0 else 0
                        lo_br = 1 - up_br
                        if th <= 0:
                            slots.append((int(r), up_br, None, False))
                        elif th >= ZS:
                            slots.append((int(r), lo_br, None, False))
                        else:
                            slots.append((int(r), lo_br, th, False))
                            slots.append((int(r), up_br, th, True))
                if slots:
                    entries.append((int(col), int(zb), slots))
        # first-fit-decreasing into matmuls
        entries.sort(key=lambda e: -len(e[2]))
        mm = []          # (nents, nslots, [entry])
        for ent in entries:
            k2 = len(ent[2])
            placed = False
            for t in mm:
                if t[0] < EPM and t[1] + k2 <= SPM:
                    t[2].append(ent)
                    t[0] += 1
                    t[1] += k2
                    placed = True
                    break
            if not placed:
                mm.append([1, k2, [ent]])
        assert len(mm) <= NM, f"core {k}: {len(mm)} matmuls > {NM}"
        plans.append([t[2] for t in mm])
    return plans


def _prep_inputs(ray_origin, ray_dir, memory):
    import ml_dtypes
    assign, lo, hi, (A0, B0, C0, B1, C1), alpha = _analyze(ray_origin, ray_dir)
    dz = ray_dir.astype(np.float64)[:, 2]
    plans = _plan(assign, lo, hi, alpha, dz)
    zaug = _zaug_rows()
    mem = np.ascontiguousarray(memory, dtype=np.float32).reshape(NCH, D, C)
    mem_bf = mem.astype(ml_dtypes.bfloat16)
    base_col = np.zeros(R)
    base_col[0] = WNEG
    base_col = _bf16(base_col)
    WNEGb = float(_bf16(WNEG))  # -29952, exactly representable

    in_maps = []
    extracts = []
    for k in range(NCORES):
        # coef [96, 512]: bank b = slot cols [sum(BANKW[:b]), +BANKW[b]) at
        # partitions 32b..32b+R-1
        coef = np.zeros((96, 512), ml_dtypes.bfloat16)
        for b in range(3):
            coef[32 * b:32 * b + R, 0:BANKW[b]] = base_col[:, None]
        memg = np.zeros((128, NM * 128), ml_dtypes.bfloat16)
        ext_row, ext_col, ext_ray = [], [], []
        for m, ents in enumerate(plans[k]):
            snext = 0
            for ei, (col, zb, slots) in enumerate(ents):
                q, j = divmod(ei, NJ)
                memg[ZS * q:ZS * (q + 1),
                     128 * m + 16 * j:128 * m + 16 * (j + 1)] = \
                    mem_bf[col, zb:zb + ZS]
                n = len(slots)
                rs = np.array([s[0] for s in slots])
                brs = np.array([s[1] for s in slots])
                Bq = np.where(brs == 0, B0[col, rs], B1[col, rs])
                Cq = np.where(brs == 0, C0[col, rs], C1[col, rs])
                # resurrect slots: add WNEG to the constant term
                res = np.array([s[3] for s in slots])
                Cq = Cq + np.where(res, WNEGb, 0.0)
                cc = _pack_cols(A0[col, rs], Bq, Cq, np.full(n, zb))
                bnk, ci = divmod(SPM * m + snext, 512)
                po = 32 * bnk
                coef[po:po + RQUAD, ci:ci + n] = cc
                # q-indicator rows: 0 on own q, WNEG elsewhere
                ind = np.full((RIND, n), WNEGb, ml_dtypes.bfloat16)
                ind[q, :] = 0.0
                coef[po + RQUAD:po + RQUAD + RIND, ci:ci + n] = ind
                # step rows
                for si, (r, br, th, rsr) in enumerate(slots):
                    if th is not None:
                        coef[po + RQUAD + RIND + th - 1, ci + si] = \
                            -WNEGb if rsr else WNEGb
                p = m // 2
                ext_row += list(64 * (m % 2) + snext + np.arange(n))
                ext_col += [128 * p + 16 * j] * n
                ext_ray += list(rs)
                snext += n
            assert snext <= SPM
        zc = np.zeros((128, 640), ml_dtypes.bfloat16)
        zc[:, 0:128] = zaug
        zc[0:96, 128:640] = coef
        in_maps.append({"zc": zc, "mem": memg})
        extracts.append((np.array(ext_row, np.int64),
                         np.array(ext_col, np.int64),
                         np.array(ext_ray, np.int64)))
    return in_maps, extracts


def _extract(results, extracts):
    out = np.zeros((B, C), np.float64)
    r16 = np.arange(16)
    for res, (row, col, ray) in zip(results, extracts):
        ps = res["out"].astype(np.float64)
        if len(row):
            vals = ps[row[:, None], col[:, None] + r16[None, :]]
            np.add.at(out, ray, vals)
    return out.astype(np.float32)


def emulate(ray_origin, ray_dir, memory):
    """Numpy emulation of the device program (packing/index validation)."""
    in_maps, extracts = _prep_inputs(ray_origin, ray_dir, memory)
    results = []
    for im in in_maps:
        zaugd = im["zc"][:, 0:128].astype(np.float64)
        coef = im["zc"][0:96, 128:640].astype(np.float64)
        psW = np.concatenate(
            [zaugd[32 * b:32 * b + R, :].T
             @ coef[32 * b:32 * b + R, 0:BANKW[b]]
             for b in range(3)], axis=1)         # [128, NM*64]
        kern = _bf16(np.exp(np.minimum(psW, 60.0))).astype(np.float64)
        out = np.zeros((128, PB * 128), np.float64)
        memg = im["mem"].astype(np.float64)
        for m in range(NM):
            blk = kern[:, SPM * m:SPM * (m + 1)].T @ \
                memg[:, 128 * m:128 * (m + 1)]          # [64, 128]
            p = m // 2
            out[64 * (m % 2):64 * (m % 2) + 64,
                128 * p:128 * (p + 1)] = blk
        results.append({"out": _bf16(out)})
    return _extract(results, extracts)


def run_kernel(ray_origin, ray_dir, memory, trace=False, **run_kwargs):
    """Run on 8 NeuronCores; returns ([B,C] output, BassKernelResults)."""
    from concourse.bass_utils import run_bass_kernel_spmd
    nc = _get_nc()
    in_maps, extracts = _prep_inputs(np.asarray(ray_origin),
                                     np.asarray(ray_dir),
                                     np.asarray(memory))
    br = run_bass_kernel_spmd(nc, in_maps, core_ids=list(range(NCORES)),
                              trace=trace, **run_kwargs)
    return _extract(br.results, extracts), br


def kernel(ray_origin, ray_dir, memory):
    out, _ = run_kernel(np.asarray(ray_origin), np.asarray(ray_dir),
                        np.asarray(memory))
    return out


# revision 3
# speedup vs baseline: 1.1991x; 1.1991x over previous
"""Trainium2 Bass kernel for the HPM gaussian-ray read problem (sparse).

out[b,c] = sum_n exp(-r2[n,b]/(2*sigma^2)) * exp(-max(t[n,b],0)/tau) * mem[n,c]

over the flattened 128^3 grid, B=32 rays, C=16 channels.

Sparse 16-z-entry design with fully batched matmuls (raw bass, manual
semaphores -- no TileContext, to minimize the fixed barrier overhead).

Host classifies each (column, ray) pair (inactive / branch W0 / branch
W1 / straddle) from the exact f64 log-weight quadratics.  Active columns
become *entries* (column, zb) covering a 16-z window; entries and their
pair-slots pack into NM=22 batched matmuls; each matmul m computes

  mm2_m: psO[64 slots, 128] = kern_m[128, 64].T @ mem_m[128, 128]

where the 128-row contraction is 8 sub-tiles x 16 z rows and the 128
moving cols are 8 entry positions x 16 channels.  kern_m is made
block-diagonal (slot s sees only its entry's 16 z rows) through 8
indicator rows in the basis: coef adds WNEG (-> exp == 0) on the 7
wrong sub-tiles.  Straddle pairs use 15 step-basis rows [z >= k] to
split exactly at the t=0 kink into a W0-slot and a W1-slot, so there is
no separate straddle path at all.

Basis (31 rows): [C0,C1,C2, B0,B1,B2, A0,A1] split-bf16 quadratic rows
(~24 mantissa bits), 8 q-indicator rows, 15 z-step rows.

Device program per core (all cores run the same static program):
  - one zc DMA [128, 640] = zaug basis + coef (sync ring; sole mm1 gate),
    mem [128, NM*128] bf16 on gpsimd (2 chunks) + scalar rings
  - mm1 x3: psW bank b [128, <=512] = zaug_b.T @ coef_b, each on its own
    PE row-tile (tile_position (32b, 0)) so they pipeline
  - Exp x3 (ACT): kern bank = exp(psW bank) (bf16)
  - mm2 x NM into 3 psO PSUM banks, alternating tile_position (0,0)/(0,64);
    psO banks are only read after ALL their matmuls are done (reading a
    PSUM bank while the PE still writes other columns wedges hardware)
  - 3 full-bank casts psO -> outbuf (bf16, DVE/ACT) + 3 out DMAs on
    sync/gpsimd/scalar rings
Host gathers each slot's 16-channel row and scatter-adds into out[B,C].

Measured: ~15.3-18.9 us HW exec (mean ~16.3) vs the 28.4-29.8 us
32-z-tile baseline; rel_l2 error 3.39e-3 (tolerance 2e-2).
"""

import numpy as np

SIGMA = 0.5
TAU = 2.0
NCORES = 8
D = 128           # grid edge
B = 32            # rays
C = 16            # channels
NCH = D * D       # 16384 (gx,gy) columns

ZS = 16           # z rows per entry
NQ = 8            # sub-tiles (z blocks) per matmul
NJ = 8            # entry positions per sub-tile... (shared col space: 8 j's)
EPM = 64          # entry positions per matmul (8 q x 8 j)
SPM = 64          # slots per matmul
NM = 22           # matmuls per core (static capacity; psW banks 512/512/384)
PB = (NM + 1) // 2            # psO pair-blocks (11)
BANKW = [512, 512, 384]       # slot cols per psW/coef bank
RQUAD = 8                     # quad basis rows
RIND = 8                      # q-indicator rows
RSTEP = 15                    # z-step rows [z >= k], k=1..15
R = RQUAD + RIND + RSTEP      # 31 basis rows
EPS = 1e-4        # per-pair neglected-mass threshold
YTHR = 1e-8       # per-z weight threshold for z-windows
WNEG = -30000.0   # "minus infinity" log-weight (bf16 exact: -29952)

_BASS_CACHE = {}


# ---------------------------------------------------------------- device ---

def _build_nc():
    from contextlib import ExitStack
    import concourse.bacc as bacc
    import concourse.mybir as mybir

    f32 = mybir.dt.float32
    bf16 = mybir.dt.bfloat16
    nc = bacc.Bacc()
    # coef: basis rows of psW bank b live at partitions 32b..32b+R-1,
    # cols 0..511 are bank b's slot columns.  zaug: 4 stacked copies of
    # the [R, 128] basis at partition offsets 0/32/64/96.  128-partition
    # layouts keep the DMA descriptors spread across all engines.
    # zc: cols 0:128 = zaug basis (4 stacked copies), cols 128:640 = coef
    # (bank b rows at partitions 32b..32b+R-1) -- one DMA, one gate
    zc_d = nc.dram_tensor("zc", [128, 640], bf16, kind="ExternalInput")
    mem_d = nc.dram_tensor("mem", [128, NM * 128], bf16, kind="ExternalInput")
    out_d = nc.dram_tensor("out", [128, PB * 128], bf16, kind="ExternalOutput")
    Exp = mybir.ActivationFunctionType.Exp

    MEMC = NM * 128                        # 3072

    with ExitStack() as ctx:
        zcsb = ctx.enter_context(nc.sbuf_tensor("zcsb", [128, 640], bf16))
        memsb = ctx.enter_context(nc.sbuf_tensor("memsb", [128, MEMC], bf16))
        kern = ctx.enter_context(nc.sbuf_tensor("kern", [128, NM * SPM],
                                                bf16))
        outbuf = ctx.enter_context(
            nc.sbuf_tensor("outbuf", [128, PB * 128], bf16))
        psw = [ctx.enter_context(
            nc.psum_tensor(f"psw{b}", [128, BANKW[b]], f32))
            for b in range(3)]
        PSOW = [512, 512, PB * 128 - 1024]
        pso = [ctx.enter_context(
            nc.psum_tensor(f"pso{b}", [128, PSOW[b]], f32))
            for b in range(3)]

        s_zc = nc.alloc_semaphore("s_zc")
        s_m = [nc.alloc_semaphore(f"s_m{i}") for i in range(3)]
        s_w = nc.alloc_semaphore("s_w")
        s_k = nc.alloc_semaphore("s_k")
        s_pso = nc.alloc_semaphore("s_pso")
        s_cpv = nc.alloc_semaphore("s_cpv")
        s_fin = [nc.alloc_semaphore(f"s_fin{i}") for i in range(3)]
        s_cps = nc.alloc_semaphore("s_cps")

        # --- DMA issue (async; per-ring in-order) ---
        # sync: one zaug+coef transfer (mm1 gate); gpsimd/scalar: memory
        nc.sync.dma_start(out=zcsb[:], in_=zc_d[:, :]).then_inc(s_zc, 16)
        nc.gpsimd.dma_start(out=memsb[:, 0:1024],
                            in_=mem_d[:, 0:1024]).then_inc(s_m[0], 16)
        nc.gpsimd.dma_start(out=memsb[:, 1024:2048],
                            in_=mem_d[:, 1024:2048]).then_inc(s_m[1], 16)
        nc.scalar.dma_start(out=memsb[:, 2048:MEMC],
                            in_=mem_d[:, 2048:MEMC]).then_inc(s_m[2], 16)

        # --- tensor: 3 x mm1 (separate PE row-tiles), 22 x mm2 ---
        nc.tensor.wait_ge(s_zc, 16)
        for b in range(3):
            nc.tensor.matmul(psw[b][:, 0:BANKW[b]],
                             zcsb[32 * b:32 * b + R, 0:128],
                             zcsb[32 * b:32 * b + R, 128:128 + BANKW[b]],
                             start=True, stop=True,
                             tile_position=(32 * b, 0)).then_inc(s_w, 1)
        # s_pso ticks after each psO bank completes (full-bank casts only:
        # reading a PSUM bank while the PE still writes other columns of
        # the same bank wedges real hardware)
        PSO_TICKS = (7, 15, 21)
        for m in range(NM):
            if m == 0:
                nc.tensor.wait_ge(s_k, 1)
                nc.tensor.wait_ge(s_m[0], 16)
            elif m == 8:
                nc.tensor.wait_ge(s_k, 2)
                nc.tensor.wait_ge(s_m[1], 16)
            elif m == 16:
                nc.tensor.wait_ge(s_k, 3)
                nc.tensor.wait_ge(s_m[2], 16)
            p = m // 2
            r0 = 64 * (m % 2)
            c0 = 128 * (p % 4)
            mm = nc.tensor.matmul(
                pso[p // 4][r0:r0 + 64, c0:c0 + 128],
                kern[:, SPM * m:SPM * (m + 1)],
                memsb[:, 128 * m:128 * (m + 1)],
                start=True, stop=True,
                tile_position=(0, r0))
            if m in PSO_TICKS:
                mm.then_inc(s_pso, 1)

        # --- scalar (ACT): 3 x Exp, then bank-2 casts + its out DMA ---
        co = 0
        for b in range(3):
            nc.scalar.wait_ge(s_w, b + 1)
            nc.scalar.activation(kern[:, co:co + BANKW[b]],
                                 psw[b][:, 0:BANKW[b]], Exp).then_inc(s_k, 1)
            co += BANKW[b]
        nc.scalar.wait_ge(s_pso, 3)
        nc.scalar.copy(out=outbuf[:, 1024:1408],
                       in_=pso[2][:, 0:384]).then_inc(s_cps, 1)
        nc.scalar.wait_ge(s_cps, 1)
        nc.scalar.dma_start(out=out_d[:, 1024:1408],
                            in_=outbuf[:, 1024:1408]).then_inc(s_fin[2], 16)

        # --- vector (DVE): full-bank casts of pso banks 0/1 ---
        for h in range(2):
            nc.vector.wait_ge(s_pso, h + 1)
            nc.vector.tensor_copy(
                outbuf[:, 512 * h:512 * (h + 1)],
                pso[h][:, 0:512],
            ).then_inc(s_cpv, 1)

        # --- out DMAs chase the casts on sync / gpsimd rings ---
        nc.sync.wait_ge(s_cpv, 1)
        nc.sync.dma_start(out=out_d[:, 0:512],
                          in_=outbuf[:, 0:512]).then_inc(s_fin[0], 16)
        nc.gpsimd.wait_ge(s_cpv, 2)
        nc.gpsimd.dma_start(out=out_d[:, 512:1024],
                            in_=outbuf[:, 512:1024]).then_inc(s_fin[1], 16)

        nc.compile()
    return nc


def _get_nc():
    if "nc" not in _BASS_CACHE:
        _BASS_CACHE["nc"] = _build_nc()
    return _BASS_CACHE["nc"]


# ------------------------------------------------------------------ host ---

def _bf16(x):
    import ml_dtypes
    return np.asarray(x).astype(ml_dtypes.bfloat16)


def _split3(x):
    """f64 -> three bf16 parts summing to ~24 mantissa bits of x."""
    x0 = _bf16(x).astype(np.float64)
    x1 = _bf16(x - x0).astype(np.float64)
    x2 = _bf16(x - x0 - x1).astype(np.float64)
    return x0, x1, x2


def _pack_cols(Aq, Bq, Cq, zb):
    """f64 quadratic in u_orig = z - 64 -> [8, n] bf16 split rows,
    recentered to u = z - zb - 8.  Rows: [C0,C1,C2, B0,B1,B2, A0,A1]."""
    Aq = np.asarray(Aq, np.float64)
    Bq = np.asarray(Bq, np.float64)
    Cq = np.asarray(Cq, np.float64)
    zb = np.asarray(zb, np.float64)
    cs = zb + 8.0 - 64.0
    Bt = Bq + 2 * Aq * cs
    Ct = Cq + Bq * cs + Aq * cs * cs
    C_0, C_1, C_2 = _split3(Ct)
    B_0, B_1, B_2 = _split3(Bt)
    A_0, A_1, _ = _split3(Aq)
    rows = [C_0, C_1, C_2, B_0, B_1, B_2, A_0, A_1]
    return np.stack([_bf16(r) for r in rows])


def _zaug_rows():
    """Basis [128, 128]: 4 stacked copies (partition offsets 0/32/64/96)
    of the [31, 128] basis over {q,z} cols: quad rows on u = z - 8,
    8 q-indicator rows, 15 step rows [z >= k]."""
    u = np.arange(ZS, dtype=np.float64) - 8.0
    one = np.ones(ZS)
    quad = np.stack([one, one, one, u, u, u, u * u, u * u])      # [8, 16]
    base = np.zeros((R, 128), np.float64)
    for q in range(NQ):
        base[0:RQUAD, ZS * q:ZS * (q + 1)] = quad
        base[RQUAD + q, ZS * q:ZS * (q + 1)] = 1.0
        for k in range(1, 16):
            base[RQUAD + RIND + k - 1, ZS * q + k:ZS * (q + 1)] = 1.0
    out = np.zeros((128, 128), np.float64)
    for b in range(4):
        out[32 * b:32 * b + R] = base
    return _bf16(out)


def _analyze(ray_origin, ray_dir):
    """Quadratic coeffs (f64) + per-(col, ray) branch assignment.

    assign: 0=inactive, 1=W0 branch, 2=W1 branch, 3=straddle."""
    o = ray_origin.astype(np.float64)
    d = ray_dir.astype(np.float64)
    d2 = (d * d).sum(-1)
    kap = 2.0 - d2
    od = (o * d).sum(-1)
    g = np.arange(D, dtype=np.float64)
    gx = np.repeat(g, D)
    gy = np.tile(g, D)
    c1 = 1.0 / (2 * SIGMA ** 2)
    c3 = 1.0 / TAU
    alpha = gx[:, None] * d[None, :, 0] + gy[:, None] * d[None, :, 1] - od[None, :]
    t64 = 64.0 * d[None, :, 2] + alpha                      # [NCH, B]
    e = 64.0 - o[:, 2]
    gamma = (gx[:, None] - o[None, :, 0]) ** 2 + (gy[:, None] - o[None, :, 1]) ** 2
    A0 = np.broadcast_to((-c1 + c1 * kap * d[:, 2] ** 2)[None, :], t64.shape)
    B0 = -2 * c1 * e[None, :] + 2 * c1 * kap[None, :] * d[None, :, 2] * t64
    C0 = -c1 * (gamma + e[None, :] ** 2) + c1 * kap[None, :] * t64 ** 2
    B1 = B0 - c3 * d[None, :, 2]
    C1 = C0 - c3 * t64

    u = np.arange(D, dtype=np.float64) - 64.0
    assign = np.zeros((NCH, B), np.int8)
    lo = np.full((NCH, B), D - 1, np.int32)
    hi = np.zeros((NCH, B), np.int32)
    CH = 2048
    for s in range(0, NCH, CH):
        sl = slice(s, s + CH)
        W0 = (A0[sl, :, None] * u[None, None, :] ** 2
              + B0[sl, :, None] * u[None, None, :] + C0[sl, :, None])
        W1 = (A0[sl, :, None] * u[None, None, :] ** 2
              + B1[sl, :, None] * u[None, None, :] + C1[sl, :, None])
        y0 = np.exp(np.minimum(W0, 50.0))
        y1 = np.exp(np.minimum(W1, 50.0))
        yt = np.minimum(y0, y1)
        Ec = yt.sum(-1)
        E0 = (y0 - yt).sum(-1)
        E1 = (y1 - yt).sum(-1)
        a = np.full(Ec.shape, 3, np.int8)
        a[E1 <= EPS] = 2
        a[E0 <= EPS] = 1
        a[Ec <= EPS] = 0
        assign[sl] = a
        m = yt >= YTHR
        any_ = m.any(-1)
        lo[sl] = np.where(any_, m.argmax(-1), D - 1)
        hi[sl] = np.where(any_, D - 1 - m[:, :, ::-1].argmax(-1), 0)
    return assign, lo, hi, (A0, B0, C0, B1, C1), alpha


def _col_entries(col, rays, lo, hi):
    """(col, zb, rays-subset) entries with disjoint 16-z coverage."""
    clo = int(lo[col, rays].min())
    chi = int(hi[col, rays].max())
    if chi - clo + 1 <= ZS:
        zb = min(clo, D - ZS)
        return [(col, zb, list(rays))]
    out = []
    for k in range(clo // ZS, chi // ZS + 1):
        rs = [r for r in rays
              if lo[col, r] < ZS * (k + 1) and hi[col, r] >= ZS * k]
        if rs:
            out.append((col, ZS * k, rs))
    return out


def _plan(assign, lo, hi, alpha, dz):
    """Columns -> cores, entries -> matmuls.

    Returns per-core list of matmuls; each matmul is a list of
    (col, zb, ent_idx, [slots]) where each slot is
    (ray, branch, theta, resurrect):
      branch 0 -> (B0, C0);  branch 1 -> (B1, C1)
      theta None -> plain slot;  else step at z >= theta within the entry,
      resurrect False -> slot active on z < theta (kill upper),
      resurrect True  -> active on z >= theta (kill lower).
    """
    act = assign != 0
    # per-column slot load (straddle pairs may need 2 slots)
    slots_per_col = ((assign == 1) | (assign == 2)).sum(1) \
        + 2 * (assign == 3).sum(1)
    cols = np.nonzero(act.any(1))[0]
    order = cols[np.argsort(-slots_per_col[cols], kind="stable")]
    loads = np.zeros(NCORES, np.int64)
    colcnt = np.zeros(NCORES, np.int64)
    core_cols = [[] for _ in range(NCORES)]
    for col in order:
        k = int(np.lexsort((colcnt, loads))[0])
        core_cols[k].append(col)
        loads[k] += slots_per_col[col]
        colcnt[k] += 1

    plans = []
    for k in range(NCORES):
        entries = []     # (col, zb, [slotspec])
        for col in sorted(core_cols[k]):
            rays = np.nonzero(act[col])[0]
            for ecol, zb, rs in _col_entries(col, rays, lo, hi):
                slots = []
                for r in rs:
                    a = assign[col, r]
                    if a == 1:
                        slots.append((int(r), 0, None, False))
                    elif a == 2:
                        slots.append((int(r), 1, None, False))
                    else:
                        # straddle: split at the t=0 kink z* = -alpha/dz
                        dzr = dz[r]
                        zstar = -alpha[col, r] / dzr if dzr != 0 else 1e9
                        zz = zstar - zb
                        th = int(np.ceil(zz))
                        # upper region z >= z*: t>0 (W1) if dz>0 else t<=0
                        up_br = 1 if dzr > 0 else 0
                        lo_br = 1 - up_br
                        if th <= 0:
                            slots.append((int(r), up_br, None, False))
                        elif th >= ZS:
                            slots.append((int(r), lo_br, None, False))
                        else:
                            slots.append((int(r), lo_br, th, False))
                            slots.append((int(r), up_br, th, True))
                if slots:
                    entries.append((int(col), int(zb), slots))
        # first-fit-decreasing into matmuls
        entries.sort(key=lambda e: -len(e[2]))
        mm = []          # (nents, nslots, [entry])
        for ent in entries:
            k2 = len(ent[2])
            placed = False
            for t in mm:
                if t[0] < EPM and t[1] + k2 <= SPM:
                    t[2].append(ent)
                    t[0] += 1
                    t[1] += k2
                    placed = True
                    break
            if not placed:
                mm.append([1, k2, [ent]])
        assert len(mm) <= NM, f"core {k}: {len(mm)} matmuls > {NM}"
        plans.append([t[2] for t in mm])
    return plans


def _prep_inputs(ray_origin, ray_dir, memory):
    import ml_dtypes
    assign, lo, hi, (A0, B0, C0, B1, C1), alpha = _analyze(ray_origin, ray_dir)
    dz = ray_dir.astype(np.float64)[:, 2]
    plans = _plan(assign, lo, hi, alpha, dz)
    zaug = _zaug_rows()
    mem = np.ascontiguousarray(memory, dtype=np.float32).reshape(NCH, D, C)
    mem_bf = mem.astype(ml_dtypes.bfloat16)
    base_col = np.zeros(R)
    base_col[0] = WNEG
    base_col = _bf16(base_col)
    WNEGb = float(_bf16(WNEG))  # -29952, exactly representable

    in_maps = []
    extracts = []
    for k in range(NCORES):
        # coef [96, 512]: bank b = slot cols [sum(BANKW[:b]), +BANKW[b]) at
        # partitions 32b..32b+R-1
        coef = np.zeros((96, 512), ml_dtypes.bfloat16)
        for b in range(3):
            coef[32 * b:32 * b + R, 0:BANKW[b]] = base_col[:, None]
        memg = np.zeros((128, NM * 128), ml_dtypes.bfloat16)
        ext_row, ext_col, ext_ray = [], [], []
        for m, ents in enumerate(plans[k]):
            snext = 0
            for ei, (col, zb, slots) in enumerate(ents):
                q, j = divmod(ei, NJ)
                memg[ZS * q:ZS * (q + 1),
                     128 * m + 16 * j:128 * m + 16 * (j + 1)] = \
                    mem_bf[col, zb:zb + ZS]
                n = len(slots)
                rs = np.array([s[0] for s in slots])
                brs = np.array([s[1] for s in slots])
                Bq = np.where(brs == 0, B0[col, rs], B1[col, rs])
                Cq = np.where(brs == 0, C0[col, rs], C1[col, rs])
                # resurrect slots: add WNEG to the constant term
                res = np.array([s[3] for s in slots])
                Cq = Cq + np.where(res, WNEGb, 0.0)
                cc = _pack_cols(A0[col, rs], Bq, Cq, np.full(n, zb))
                bnk, ci = divmod(SPM * m + snext, 512)
                po = 32 * bnk
                coef[po:po + RQUAD, ci:ci + n] = cc
                # q-indicator rows: 0 on own q, WNEG elsewhere
                ind = np.full((RIND, n), WNEGb, ml_dtypes.bfloat16)
                ind[q, :] = 0.0
                coef[po + RQUAD:po + RQUAD + RIND, ci:ci + n] = ind
                # step rows
                for si, (r, br, th, rsr) in enumerate(slots):
                    if th is not None:
                        coef[po + RQUAD + RIND + th - 1, ci + si] = \
                            -WNEGb if rsr else WNEGb
                p = m // 2
                ext_row += list(64 * (m % 2) + snext + np.arange(n))
                ext_col += [128 * p + 16 * j] * n
                ext_ray += list(rs)
                snext += n
            assert snext <= SPM
        zc = np.zeros((128, 640), ml_dtypes.bfloat16)
        zc[:, 0:128] = zaug
        zc[0:96, 128:640] = coef
        in_maps.append({"zc": zc, "mem": memg})
        extracts.append((np.array(ext_row, np.int64),
                         np.array(ext_col, np.int64),
                         np.array(ext_ray, np.int64)))
    return in_maps, extracts


def _extract(results, extracts):
    out = np.zeros((B, C), np.float64)
    r16 = np.arange(16)
    for res, (row, col, ray) in zip(results, extracts):
        ps = res["out"].astype(np.float64)
        if len(row):
            vals = ps[row[:, None], col[:, None] + r16[None, :]]
            np.add.at(out, ray, vals)
    return out.astype(np.float32)


def emulate(ray_origin, ray_dir, memory):
    """Numpy emulation of the device program (packing/index validation)."""
    in_maps, extracts = _prep_inputs(ray_origin, ray_dir, memory)
    results = []
    for im in in_maps:
        zaugd = im["zc"][:, 0:128].astype(np.float64)
        coef = im["zc"][0:96, 128:640].astype(np.float64)
        psW = np.concatenate(
            [zaugd[32 * b:32 * b + R, :].T
             @ coef[32 * b:32 * b + R, 0:BANKW[b]]
             for b in range(3)], axis=1)         # [128, NM*64]
        kern = _bf16(np.exp(np.minimum(psW, 60.0))).astype(np.float64)
        out = np.zeros((128, PB * 128), np.float64)
        memg = im["mem"].astype(np.float64)
        for m in range(NM):
            blk = kern[:, SPM * m:SPM * (m + 1)].T @ \
                memg[:, 128 * m:128 * (m + 1)]          # [64, 128]
            p = m // 2
            out[64 * (m % 2):64 * (m % 2) + 64,
                128 * p:128 * (p + 1)] = blk
        results.append({"out": _bf16(out)})
    return _extract(results, extracts)


def run_kernel(ray_origin, ray_dir, memory, trace=False, **run_kwargs):
    """Run on 8 NeuronCores; returns ([B,C] output, BassKernelResults)."""
    from concourse.bass_utils import run_bass_kernel_spmd
    nc = _get_nc()
    in_maps, extracts = _prep_inputs(np.asarray(ray_origin),
                                     np.asarray(ray_dir),
                                     np.asarray(memory))
    br = run_bass_kernel_spmd(nc, in_maps, core_ids=list(range(NCORES)),
                              trace=trace, **run_kwargs)
    return _extract(br.results, extracts), br


def kernel(ray_origin, ray_dir, memory):
    out, _ = run_kernel(np.asarray(ray_origin), np.asarray(ray_dir),
                        np.asarray(memory))
    return out


# revision 5
# speedup vs baseline: 1.2014x; 1.0019x over previous
"""Trainium2 Bass kernel for the HPM gaussian-ray read problem (sparse).

out[b,c] = sum_n exp(-r2[n,b]/(2*sigma^2)) * exp(-max(t[n,b],0)/tau) * mem[n,c]

over the flattened 128^3 grid, B=32 rays, C=16 channels.

16-z-entry sparse design, fully batched matmuls, raw bass with manual
semaphores (no TileContext) to minimize fixed barrier overhead.

Host classifies each (column, ray) pair (inactive / branch W0 / branch W1 /
straddle) from exact f64 log-weight quadratics.  Active columns are grouped
into *entries* (column, zb) covering a 16-z window.  Entries and their
pair-slots are packed into NM=22 batched matmuls; each matmul m:

  mm2_m: psO[64 slots, 128] = kern_m[128, 64].T @ mem_m[128, 128]

where the 128-row contraction dim is 8 sub-tiles x 16 z rows, and the
128 moving cols are 8 entry positions x 16 channels.  kern_m is made
block-diagonal (slot s only sees its entry's 16 z rows) through 8
indicator rows in the basis: coef adds WNEG (-> exp == 0) on the 7 wrong
sub-tiles.  Straddle pairs use 15 extra step-basis rows [z >= k] to
split exactly at the t=0 kink into a W0-slot and a W1-slot, so there is
no separate straddle path at all.

Basis (31 rows): [C0,C1,C2, B0,B1,B2, A0,A1] split-bf16 quadratic rows,
8 q-indicator rows, 15 z-step rows.

Device program per core (all cores run the same static program):
  - one zc DMA [96, 640] = basis + coef (sync ring; sole mm1 gate);
    mem [128, NM*128] bf16 on gpsimd (2 chunks) + scalar rings
  - mm1 x3: psW bank b [128, <=512] = zaug_b.T @ coef_b, each on its own
    PE row-tile (tile_position (32b, 0)) so they pipeline
  - Exp x3 (ACT): kern bank = exp(psW bank)  (bf16)
  - mm2 x NM into 3 psO PSUM banks, alternating tile_position (0,0)/(0,64);
    a psO bank is only read after ALL its matmuls finish (reading a PSUM
    bank while the PE still writes other columns wedges real hardware)
  - 3 full-bank casts psO -> outbuf (bf16, DVE/ACT) + 3 out DMAs on
    sync/gpsimd/scalar rings
Host gathers the per-slot 16-channel rows and scatter-adds into out[B,C].

Measured: ~15.8-16.1 us HW exec vs the 28.4-29.8 us 32-z-tile baseline
(~1.8x); rel_l2 error 3.39e-3 (tolerance 2e-2).
"""

import numpy as np

SIGMA = 0.5
TAU = 2.0
NCORES = 8
D = 128           # grid edge
B = 32            # rays
C = 16            # channels
NCH = D * D       # 16384 (gx,gy) columns

ZS = 16           # z rows per entry
NQ = 8            # sub-tiles (z blocks) per matmul
NJ = 8            # entry positions per sub-tile... (shared col space: 8 j's)
EPM = 64          # entry positions per matmul (8 q x 8 j)
SPM = 64          # slots per matmul
NM = 22           # matmuls per core (static capacity; psW banks 512/512/384)
PB = (NM + 1) // 2            # psO pair-blocks (11)
BANKW = [512, 512, 384]       # slot cols per psW/coef bank
RQUAD = 8                     # quad basis rows
RIND = 8                      # q-indicator rows
RSTEP = 15                    # z-step rows [z >= k], k=1..15
R = RQUAD + RIND + RSTEP      # 31 basis rows
EPS = 1e-4        # per-pair neglected-mass threshold
YTHR = 1e-8       # per-z weight threshold for z-windows
WNEG = -30000.0   # "minus infinity" log-weight (bf16 exact: -29952)

_BASS_CACHE = {}


# ---------------------------------------------------------------- device ---

def _build_nc():
    from contextlib import ExitStack
    import concourse.bacc as bacc
    import concourse.mybir as mybir

    f32 = mybir.dt.float32
    bf16 = mybir.dt.bfloat16
    nc = bacc.Bacc()
    # zc: cols 0:128 = zaug basis (3 stacked copies at partition offsets
    # 0/32/64), cols 128:640 = coef (bank b rows at partitions
    # 32b..32b+R-1).  One wide DMA: descriptors spread across all 16 DMA
    # engines, and a single semaphore gates all three mm1s.
    zc_d = nc.dram_tensor("zc", [96, 640], bf16, kind="ExternalInput")
    mem_d = nc.dram_tensor("mem", [128, NM * 128], bf16, kind="ExternalInput")
    out_d = nc.dram_tensor("out", [128, PB * 128], bf16, kind="ExternalOutput")
    Exp = mybir.ActivationFunctionType.Exp

    MEMC = NM * 128                        # 3072

    with ExitStack() as ctx:
        zcsb = ctx.enter_context(nc.sbuf_tensor("zcsb", [96, 640], bf16))
        memsb = ctx.enter_context(nc.sbuf_tensor("memsb", [128, MEMC], bf16))
        kern = ctx.enter_context(nc.sbuf_tensor("kern", [128, NM * SPM],
                                                bf16))
        outbuf = ctx.enter_context(
            nc.sbuf_tensor("outbuf", [128, PB * 128], bf16))
        psw = [ctx.enter_context(
            nc.psum_tensor(f"psw{b}", [128, BANKW[b]], f32))
            for b in range(3)]
        PSOW = [512, 512, PB * 128 - 1024]
        pso = [ctx.enter_context(
            nc.psum_tensor(f"pso{b}", [128, PSOW[b]], f32))
            for b in range(3)]

        s_zc = nc.alloc_semaphore("s_zc")
        s_m = [nc.alloc_semaphore(f"s_m{i}") for i in range(3)]
        s_w = nc.alloc_semaphore("s_w")
        s_k = nc.alloc_semaphore("s_k")
        s_pso = nc.alloc_semaphore("s_pso")
        s_cpv = nc.alloc_semaphore("s_cpv")
        s_fin = [nc.alloc_semaphore(f"s_fin{i}") for i in range(3)]
        s_cps = nc.alloc_semaphore("s_cps")

        # --- DMA issue (async; per-ring in-order) ---
        # sync: one zaug+coef transfer (mm1 gate); gpsimd/scalar: memory
        nc.sync.dma_start(out=zcsb[:], in_=zc_d[:, :]).then_inc(s_zc, 16)
        nc.gpsimd.dma_start(out=memsb[:, 0:1024],
                            in_=mem_d[:, 0:1024]).then_inc(s_m[0], 16)
        nc.gpsimd.dma_start(out=memsb[:, 1024:2048],
                            in_=mem_d[:, 1024:2048]).then_inc(s_m[1], 16)
        nc.scalar.dma_start(out=memsb[:, 2048:MEMC],
                            in_=mem_d[:, 2048:MEMC]).then_inc(s_m[2], 16)

        # --- tensor: 3 x mm1 (separate PE row-tiles), 22 x mm2 ---
        nc.tensor.wait_ge(s_zc, 16)
        for b in range(3):
            nc.tensor.matmul(psw[b][:, 0:BANKW[b]],
                             zcsb[32 * b:32 * b + R, 0:128],
                             zcsb[32 * b:32 * b + R, 128:128 + BANKW[b]],
                             start=True, stop=True,
                             tile_position=(32 * b, 0)).then_inc(s_w, 1)
        # s_pso ticks after each psO bank completes (full-bank casts only:
        # reading a PSUM bank while the PE still writes other columns of
        # the same bank wedges real hardware)
        PSO_TICKS = (7, 15, 21)
        for m in range(NM):
            if m == 0:
                nc.tensor.wait_ge(s_k, 1)
                nc.tensor.wait_ge(s_m[0], 16)
            elif m == 8:
                nc.tensor.wait_ge(s_k, 2)
                nc.tensor.wait_ge(s_m[1], 16)
            elif m == 16:
                nc.tensor.wait_ge(s_k, 3)
                nc.tensor.wait_ge(s_m[2], 16)
            p = m // 2
            r0 = 64 * (m % 2)
            c0 = 128 * (p % 4)
            mm = nc.tensor.matmul(
                pso[p // 4][r0:r0 + 64, c0:c0 + 128],
                kern[:, SPM * m:SPM * (m + 1)],
                memsb[:, 128 * m:128 * (m + 1)],
                start=True, stop=True,
                tile_position=(0, r0))
            if m in PSO_TICKS:
                mm.then_inc(s_pso, 1)

        # --- scalar (ACT): 3 x Exp, then bank-2 casts + its out DMA ---
        co = 0
        for b in range(3):
            nc.scalar.wait_ge(s_w, b + 1)
            nc.scalar.activation(kern[:, co:co + BANKW[b]],
                                 psw[b][:, 0:BANKW[b]], Exp).then_inc(s_k, 1)
            co += BANKW[b]
        nc.scalar.wait_ge(s_pso, 3)
        nc.scalar.copy(out=outbuf[:, 1024:1408],
                       in_=pso[2][:, 0:384]).then_inc(s_cps, 1)
        nc.scalar.wait_ge(s_cps, 1)
        nc.scalar.dma_start(out=out_d[:, 1024:1408],
                            in_=outbuf[:, 1024:1408]).then_inc(s_fin[2], 16)

        # --- vector (DVE): full-bank casts of pso banks 0/1 ---
        for h in range(2):
            nc.vector.wait_ge(s_pso, h + 1)
            nc.vector.tensor_copy(
                outbuf[:, 512 * h:512 * (h + 1)],
                pso[h][:, 0:512],
            ).then_inc(s_cpv, 1)

        # --- out DMAs chase the casts on sync / gpsimd rings ---
        nc.sync.wait_ge(s_cpv, 1)
        nc.sync.dma_start(out=out_d[:, 0:512],
                          in_=outbuf[:, 0:512]).then_inc(s_fin[0], 16)
        nc.gpsimd.wait_ge(s_cpv, 2)
        nc.gpsimd.dma_start(out=out_d[:, 512:1024],
                            in_=outbuf[:, 512:1024]).then_inc(s_fin[1], 16)

        nc.compile()
    return nc


def _get_nc():
    if "nc" not in _BASS_CACHE:
        _BASS_CACHE["nc"] = _build_nc()
    return _BASS_CACHE["nc"]


# ------------------------------------------------------------------ host ---

def _bf16(x):
    import ml_dtypes
    return np.asarray(x).astype(ml_dtypes.bfloat16)


def _split3(x):
    """f64 -> three bf16 parts summing to ~24 mantissa bits of x."""
    x0 = _bf16(x).astype(np.float64)
    x1 = _bf16(x - x0).astype(np.float64)
    x2 = _bf16(x - x0 - x1).astype(np.float64)
    return x0, x1, x2


def _pack_cols(Aq, Bq, Cq, zb):
    """f64 quadratic in u_orig = z - 64 -> [8, n] bf16 split rows,
    recentered to u = z - zb - 8.  Rows: [C0,C1,C2, B0,B1,B2, A0,A1]."""
    Aq = np.asarray(Aq, np.float64)
    Bq = np.asarray(Bq, np.float64)
    Cq = np.asarray(Cq, np.float64)
    zb = np.asarray(zb, np.float64)
    cs = zb + 8.0 - 64.0
    Bt = Bq + 2 * Aq * cs
    Ct = Cq + Bq * cs + Aq * cs * cs
    C_0, C_1, C_2 = _split3(Ct)
    B_0, B_1, B_2 = _split3(Bt)
    A_0, A_1, _ = _split3(Aq)
    rows = [C_0, C_1, C_2, B_0, B_1, B_2, A_0, A_1]
    return np.stack([_bf16(r) for r in rows])


def _zaug_rows():
    """Basis [128, 128]: 4 stacked copies (partition offsets 0/32/64/96)
    of the [31, 128] basis over {q,z} cols: quad rows on u = z - 8,
    8 q-indicator rows, 15 step rows [z >= k]."""
    u = np.arange(ZS, dtype=np.float64) - 8.0
    one = np.ones(ZS)
    quad = np.stack([one, one, one, u, u, u, u * u, u * u])      # [8, 16]
    base = np.zeros((R, 128), np.float64)
    for q in range(NQ):
        base[0:RQUAD, ZS * q:ZS * (q + 1)] = quad
        base[RQUAD + q, ZS * q:ZS * (q + 1)] = 1.0
        for k in range(1, 16):
            base[RQUAD + RIND + k - 1, ZS * q + k:ZS * (q + 1)] = 1.0
    out = np.zeros((128, 128), np.float64)
    for b in range(4):
        out[32 * b:32 * b + R] = base
    return _bf16(out)


def _analyze(ray_origin, ray_dir):
    """Quadratic coeffs (f64) + per-(col, ray) branch assignment.

    assign: 0=inactive, 1=W0 branch, 2=W1 branch, 3=straddle."""
    o = ray_origin.astype(np.float64)
    d = ray_dir.astype(np.float64)
    d2 = (d * d).sum(-1)
    kap = 2.0 - d2
    od = (o * d).sum(-1)
    g = np.arange(D, dtype=np.float64)
    gx = np.repeat(g, D)
    gy = np.tile(g, D)
    c1 = 1.0 / (2 * SIGMA ** 2)
    c3 = 1.0 / TAU
    alpha = gx[:, None] * d[None, :, 0] + gy[:, None] * d[None, :, 1] - od[None, :]
    t64 = 64.0 * d[None, :, 2] + alpha                      # [NCH, B]
    e = 64.0 - o[:, 2]
    gamma = (gx[:, None] - o[None, :, 0]) ** 2 + (gy[:, None] - o[None, :, 1]) ** 2
    A0 = np.broadcast_to((-c1 + c1 * kap * d[:, 2] ** 2)[None, :], t64.shape)
    B0 = -2 * c1 * e[None, :] + 2 * c1 * kap[None, :] * d[None, :, 2] * t64
    C0 = -c1 * (gamma + e[None, :] ** 2) + c1 * kap[None, :] * t64 ** 2
    B1 = B0 - c3 * d[None, :, 2]
    C1 = C0 - c3 * t64

    u = np.arange(D, dtype=np.float64) - 64.0
    assign = np.zeros((NCH, B), np.int8)
    lo = np.full((NCH, B), D - 1, np.int32)
    hi = np.zeros((NCH, B), np.int32)
    CH = 2048
    for s in range(0, NCH, CH):
        sl = slice(s, s + CH)
        W0 = (A0[sl, :, None] * u[None, None, :] ** 2
              + B0[sl, :, None] * u[None, None, :] + C0[sl, :, None])
        W1 = (A0[sl, :, None] * u[None, None, :] ** 2
              + B1[sl, :, None] * u[None, None, :] + C1[sl, :, None])
        y0 = np.exp(np.minimum(W0, 50.0))
        y1 = np.exp(np.minimum(W1, 50.0))
        yt = np.minimum(y0, y1)
        Ec = yt.sum(-1)
        E0 = (y0 - yt).sum(-1)
        E1 = (y1 - yt).sum(-1)
        a = np.full(Ec.shape, 3, np.int8)
        a[E1 <= EPS] = 2
        a[E0 <= EPS] = 1
        a[Ec <= EPS] = 0
        assign[sl] = a
        m = yt >= YTHR
        any_ = m.any(-1)
        lo[sl] = np.where(any_, m.argmax(-1), D - 1)
        hi[sl] = np.where(any_, D - 1 - m[:, :, ::-1].argmax(-1), 0)
    return assign, lo, hi, (A0, B0, C0, B1, C1), alpha


def _col_entries(col, rays, lo, hi):
    """(col, zb, rays-subset) entries with disjoint 16-z coverage."""
    clo = int(lo[col, rays].min())
    chi = int(hi[col, rays].max())
    if chi - clo + 1 <= ZS:
        zb = min(clo, D - ZS)
        return [(col, zb, list(rays))]
    out = []
    for k in range(clo // ZS, chi // ZS + 1):
        rs = [r for r in rays
              if lo[col, r] < ZS * (k + 1) and hi[col, r] >= ZS * k]
        if rs:
            out.append((col, ZS * k, rs))
    return out


def _plan(assign, lo, hi, alpha, dz):
    """Columns -> cores, entries -> matmuls.

    Returns per-core list of matmuls; each matmul is a list of
    (col, zb, ent_idx, [slots]) where each slot is
    (ray, branch, theta, resurrect):
      branch 0 -> (B0, C0);  branch 1 -> (B1, C1)
      theta None -> plain slot;  else step at z >= theta within the entry,
      resurrect False -> slot active on z < theta (kill upper),
      resurrect True  -> active on z >= theta (kill lower).
    """
    act = assign != 0
    # per-column slot load (straddle pairs may need 2 slots)
    slots_per_col = ((assign == 1) | (assign == 2)).sum(1) \
        + 2 * (assign == 3).sum(1)
    cols = np.nonzero(act.any(1))[0]
    order = cols[np.argsort(-slots_per_col[cols], kind="stable")]
    loads = np.zeros(NCORES, np.int64)
    colcnt = np.zeros(NCORES, np.int64)
    core_cols = [[] for _ in range(NCORES)]
    for col in order:
        k = int(np.lexsort((colcnt, loads))[0])
        core_cols[k].append(col)
        loads[k] += slots_per_col[col]
        colcnt[k] += 1

    plans = []
    for k in range(NCORES):
        entries = []     # (col, zb, [slotspec])
        for col in sorted(core_cols[k]):
            rays = np.nonzero(act[col])[0]
            for ecol, zb, rs in _col_entries(col, rays, lo, hi):
                slots = []
                for r in rs:
                    a = assign[col, r]
                    if a == 1:
                        slots.append((int(r), 0, None, False))
                    elif a == 2:
                        slots.append((int(r), 1, None, False))
                    else:
                        # straddle: split at the t=0 kink z* = -alpha/dz
                        dzr = dz[r]
                        zstar = -alpha[col, r] / dzr if dzr != 0 else 1e9
                        zz = zstar - zb
                        th = int(np.ceil(zz))
                        # upper region z >= z*: t>0 (W1) if dz>0 else t<=0
                        up_br = 1 if dzr > 0 else 0
                        lo_br = 1 - up_br
                        if th <= 0:
                            slots.append((int(r), up_br, None, False))
                        elif th >= ZS:
                            slots.append((int(r), lo_br, None, False))
                        else:
                            slots.append((int(r), lo_br, th, False))
                            slots.append((int(r), up_br, th, True))
                if slots:
                    entries.append((int(col), int(zb), slots))
        # first-fit-decreasing into matmuls
        entries.sort(key=lambda e: -len(e[2]))
        mm = []          # (nents, nslots, [entry])
        for ent in entries:
            k2 = len(ent[2])
            placed = False
            for t in mm:
                if t[0] < EPM and t[1] + k2 <= SPM:
                    t[2].append(ent)
                    t[0] += 1
                    t[1] += k2
                    placed = True
                    break
            if not placed:
                mm.append([1, k2, [ent]])
        assert len(mm) <= NM, f"core {k}: {len(mm)} matmuls > {NM}"
        plans.append([t[2] for t in mm])
    return plans


def _prep_inputs(ray_origin, ray_dir, memory):
    import ml_dtypes
    assign, lo, hi, (A0, B0, C0, B1, C1), alpha = _analyze(ray_origin, ray_dir)
    dz = ray_dir.astype(np.float64)[:, 2]
    plans = _plan(assign, lo, hi, alpha, dz)
    zaug = _zaug_rows()
    mem = np.ascontiguousarray(memory, dtype=np.float32).reshape(NCH, D, C)
    mem_bf = mem.astype(ml_dtypes.bfloat16)
    base_col = np.zeros(R)
    base_col[0] = WNEG
    base_col = _bf16(base_col)
    WNEGb = float(_bf16(WNEG))  # -29952, exactly representable

    in_maps = []
    extracts = []
    for k in range(NCORES):
        # coef [96, 512]: bank b = slot cols [sum(BANKW[:b]), +BANKW[b]) at
        # partitions 32b..32b+R-1
        coef = np.zeros((96, 512), ml_dtypes.bfloat16)
        for b in range(3):
            coef[32 * b:32 * b + R, 0:BANKW[b]] = base_col[:, None]
        memg = np.zeros((128, NM * 128), ml_dtypes.bfloat16)
        ext_row, ext_col, ext_ray = [], [], []
        for m, ents in enumerate(plans[k]):
            snext = 0
            for ei, (col, zb, slots) in enumerate(ents):
                q, j = divmod(ei, NJ)
                memg[ZS * q:ZS * (q + 1),
                     128 * m + 16 * j:128 * m + 16 * (j + 1)] = \
                    mem_bf[col, zb:zb + ZS]
                n = len(slots)
                rs = np.array([s[0] for s in slots])
                brs = np.array([s[1] for s in slots])
                Bq = np.where(brs == 0, B0[col, rs], B1[col, rs])
                Cq = np.where(brs == 0, C0[col, rs], C1[col, rs])
                # resurrect slots: add WNEG to the constant term
                res = np.array([s[3] for s in slots])
                Cq = Cq + np.where(res, WNEGb, 0.0)
                cc = _pack_cols(A0[col, rs], Bq, Cq, np.full(n, zb))
                bnk, ci = divmod(SPM * m + snext, 512)
                po = 32 * bnk
                coef[po:po + RQUAD, ci:ci + n] = cc
                # q-indicator rows: 0 on own q, WNEG elsewhere
                ind = np.full((RIND, n), WNEGb, ml_dtypes.bfloat16)
                ind[q, :] = 0.0
                coef[po + RQUAD:po + RQUAD + RIND, ci:ci + n] = ind
                # step rows
                for si, (r, br, th, rsr) in enumerate(slots):
                    if th is not None:
                        coef[po + RQUAD + RIND + th - 1, ci + si] = \
                            -WNEGb if rsr else WNEGb
                p = m // 2
                ext_row += list(64 * (m % 2) + snext + np.arange(n))
                ext_col += [128 * p + 16 * j] * n
                ext_ray += list(rs)
                snext += n
            assert snext <= SPM
        zc = np.zeros((96, 640), ml_dtypes.bfloat16)
        zc[:, 0:128] = zaug[0:96]
        zc[0:96, 128:640] = coef
        in_maps.append({"zc": zc, "mem": memg})
        extracts.append((np.array(ext_row, np.int64),
                         np.array(ext_col, np.int64),
                         np.array(ext_ray, np.int64)))
    return in_maps, extracts


def _extract(results, extracts):
    out = np.zeros((B, C), np.float64)
    r16 = np.arange(16)
    for res, (row, col, ray) in zip(results, extracts):
        ps = res["out"].astype(np.float64)
        if len(row):
            vals = ps[row[:, None], col[:, None] + r16[None, :]]
            np.add.at(out, ray, vals)
    return out.astype(np.float32)


def emulate(ray_origin, ray_dir, memory):
    """Numpy emulation of the device program (packing/index validation)."""
    in_maps, extracts = _prep_inputs(ray_origin, ray_dir, memory)
    results = []
    for im in in_maps:
        zaugd = im["zc"][:, 0:128].astype(np.float64)
        coef = im["zc"][0:96, 128:640].astype(np.float64)
        psW = np.concatenate(
            [zaugd[32 * b:32 * b + R, :].T
             @ coef[32 * b:32 * b + R, 0:BANKW[b]]
             for b in range(3)], axis=1)         # [128, NM*64]
        kern = _bf16(np.exp(np.minimum(psW, 60.0))).astype(np.float64)
        out = np.zeros((128, PB * 128), np.float64)
        memg = im["mem"].astype(np.float64)
        for m in range(NM):
            blk = kern[:, SPM * m:SPM * (m + 1)].T @ \
                memg[:, 128 * m:128 * (m + 1)]          # [64, 128]
            p = m // 2
            out[64 * (m % 2):64 * (m % 2) + 64,
                128 * p:128 * (p + 1)] = blk
        results.append({"out": _bf16(out)})
    return _extract(results, extracts)


def run_kernel(ray_origin, ray_dir, memory, trace=False, **run_kwargs):
    """Run on 8 NeuronCores; returns ([B,C] output, BassKernelResults)."""
    from concourse.bass_utils import run_bass_kernel_spmd
    nc = _get_nc()
    in_maps, extracts = _prep_inputs(np.asarray(ray_origin),
                                     np.asarray(ray_dir),
                                     np.asarray(memory))
    br = run_bass_kernel_spmd(nc, in_maps, core_ids=list(range(NCORES)),
                              trace=trace, **run_kwargs)
    return _extract(br.results, extracts), br


def kernel(ray_origin, ray_dir, memory):
    out, _ = run_kernel(np.asarray(ray_origin), np.asarray(ray_dir),
                        np.asarray(memory))
    return out
